# revision 3
# baseline (speedup 1.0000x reference)
"""Trainium2 Bass kernel v2: batched forward kinematics (nn_DiffKin).

W[b, n] = prod_{i<=n} ( O_i @ M_i(angle_i(b)) )   (affine 3x4 transforms)

Design (per core, pure data-parallel across 8 cores, b_core = 8192):
  * Host folds all structure into a coefficient-row block table
    T [nrows=125, nf*12] fp16 with rows {ones, u_n, w_n} such that
       L_n(b) = sum_r coef_r(b) * T[r, n]   (3x4 incl. t-col)
    where coef rows are 1, sin(x_n), cos(x_n) (rev) or the linear angle
    (prismatic), x_n = clamp(r_n*(mult*theta+off)).
  * Device phase A (DVE+Act): build x planes, Sin/Abs/Sin -> uw
    coefficient tile [P, 125, q] fp16.
  * Phase B (PE): per 128-batch column qq: PE-transpose uw[:, :, qq]
    -> coefT [125, qq, 128] fp16, then 2 fp16 matmuls against the table
    -> L for all frames of that column -> evac PSUM->SBUF fp16
    L16 [P, nf, 12, q]. Split in frame-halves so the chain starts after
    half 0.
  * Phase C (DVE+Pool, q-sliced): sequential chain W_n = W_{n-1} @ L_n
    entirely in fp16 (DVE 2x mode), 5 tensor ops [3,4,q] + t-col add.
    Output written per frame via DMA as [nf, P, 12, q] fp16.
  * Host assembles [B, nf, 4, 4] fp32 (bottom rows + cast).

Accuracy: all-fp16 pipeline verified in numpy emulation at
rel_l2 = 2.0e-3 vs float64 reference (gate 2e-2).
"""

import os
import sys

import numpy as np

for _p in ("/opt/trn_rl_repo", "/root/.axon_site/_ro/trn_rl_repo"):
    if os.path.isdir(_p) and _p not in sys.path:
        sys.path.append(_p)

import concourse.bass as bass  # noqa: E402
import concourse.tile as tile  # noqa: E402
from concourse import bacc, masks, mybir  # noqa: E402
from concourse.bass_utils import run_bass_kernel_spmd  # noqa: E402

F32 = mybir.dt.float32
F16 = mybir.dt.float16
AF = mybir.ActivationFunctionType
OP = mybir.AluOpType

N_CORES = 8
P = 128

last_results = None
last_in_maps = None
_program_cache = {}
_host_cache = {}


# --------------------------------------------------------------------------
# Host-side folding
# --------------------------------------------------------------------------

def _skew(a):
    x, y, z = a
    return np.array([[0.0, -z, y], [z, 0.0, -x], [-y, x, 0.0]], dtype=np.float64)


def _fold(all_axes, all_origins, mimic_multipliers, mimic_offsets,
          ctrlable_indices, mimic_dst_indices, mimic_src_indices, joint_types):
    """Fold structure into (nrows, T16, multcol, extra_rows, row layout)."""
    axes = np.asarray(all_axes, dtype=np.float64)
    origins = np.asarray(all_origins, dtype=np.float64)
    nf = origins.shape[0]
    types = np.asarray(joint_types).astype(np.int64)
    ctrl = np.asarray(ctrlable_indices).astype(np.int64)
    mdst = np.asarray(mimic_dst_indices).astype(np.int64)
    msrc = np.asarray(mimic_src_indices).astype(np.int64)
    mmul = np.asarray(mimic_multipliers, dtype=np.float64)
    moff = np.asarray(mimic_offsets, dtype=np.float64)

    bottom = origins[:, 3, :]
    assert np.all(np.abs(bottom - np.array([0.0, 0.0, 0.0, 1.0])) < 1e-6), \
        "kernel v2 requires affine origins"

    dof = len(ctrl)
    # per-frame angle source: angle_n(b) = mult*theta[b, src] + off
    src = [None] * nf
    mult = [0.0] * nf
    off = [0.0] * nf
    for j, ci in enumerate(ctrl):
        src[int(ci)] = j
        mult[int(ci)] = 1.0
        off[int(ci)] = 0.0
    pre = (list(src), list(mult), list(off))
    for d, s, m, o in zip(mdst, msrc, mmul, moff):
        d, s = int(d), int(s)
        if pre[0][s] is not None:
            src[d] = pre[0][s]
            mult[d] = float(m) * pre[1][s]
            off[d] = float(m) * pre[2][s] + float(o)
        else:
            src[d] = None
            mult[d] = 0.0
            off[d] = float(o)

    # x-plane rows 0..dof-1 must be exactly theta columns scaled (no offset):
    # frames 0..dof-1 with src==n, off==0. The reference guarantees this.
    main_frames = []   # frames whose x comes from the block TT build
    extra = []         # frames needing a dedicated tensor_scalar x row
    for n in range(nf):
        if src[n] is None:
            continue
        if n < dof and src[n] == n and off[n] == 0.0 and mult[n] == 1.0:
            main_frames.append(n)
        else:
            extra.append(n)
    assert main_frames == list(range(dof)), "unexpected ctrl structure"

    # x rows: 0..dof-1 = frames 0..dof-1 (scaled by multcol);
    # dof+i = extra frame i (own tensor_scalar).
    nx = dof + len(extra)
    multcol = np.zeros(dof, dtype=np.float64)

    # clamp rows: rev frames only (their sin/cos LUT needs [-pi,pi])
    clamp_main = []    # row indices < dof to clamp (contiguous prefix check)
    # u rows: one per var frame; w rows: one per rev var frame
    rev_frames = [n for n in range(nf)
                  if src[n] is not None and types[n] == 1
                  and np.linalg.norm(axes[n]) > 1e-20]
    var_frames = [n for n in range(nf) if src[n] is not None
                  and (types[n] in (1, 2))]
    u_row = {}
    w_row = {}
    r = 1
    for n in var_frames:
        u_row[n] = r
        r += 1
    for n in rev_frames:
        w_row[n] = r
        r += 1
    nrows = r
    assert nrows <= 128, f"too many coefficient rows: {nrows}"

    xrow = {}          # frame -> x-plane row
    for n in main_frames:
        xrow[n] = n
    for i, n in enumerate(extra):
        xrow[n] = dof + i

    T = np.zeros((nrows, nf, 3, 4), dtype=np.float64)
    extra_specs = []   # (xrow, src, mult_eff, off_eff)
    for n in range(nf):
        O4 = origins[n]
        A = O4[:3, :].copy()
        t = int(types[n])
        if t == 1 and src[n] is not None:
            rr = float(np.linalg.norm(axes[n]))
            if rr < 1e-20:
                t = 0
            else:
                K4 = np.zeros((4, 4))
                K4[:3, :3] = _skew(axes[n] / rr)
                Bm = (O4 @ K4)[:3, :]
                Cm = (O4 @ K4 @ K4)[:3, :]
                T[0, n] = A + Cm
                T[u_row[n], n] = Bm
                T[w_row[n], n] = -Cm
                if n in main_frames:
                    multcol[n] = rr * mult[n]
                    clamp_main.append(n)
                else:
                    extra_specs.append((xrow[n], src[n], rr * mult[n],
                                        rr * off[n]))
                continue
        if t == 1 and src[n] is None:
            rr = float(np.linalg.norm(axes[n]))
            if rr >= 1e-20:
                a = rr * off[n]
                K4 = np.zeros((4, 4))
                K4[:3, :3] = _skew(axes[n] / rr)
                Bm = (O4 @ K4)[:3, :]
                Cm = (O4 @ K4 @ K4)[:3, :]
                T[0, n] = A + np.sin(a) * Bm + (1.0 - np.cos(a)) * Cm
            else:
                T[0, n] = A
            continue
        if t == 2:
            T4 = np.zeros((4, 4))
            T4[:3, 3] = axes[n]
            Bm = (O4 @ T4)[:3, :]
            if src[n] is None:
                T[0, n] = A + off[n] * Bm
            else:
                T[0, n] = A
                T[u_row[n], n] = Bm
                if n in main_frames:
                    multcol[n] = mult[n]
                    assert off[n] == 0.0
                else:
                    extra_specs.append((xrow[n], src[n], mult[n], off[n]))
            continue
        # fixed / degenerate
        T[0, n] = A

    # main-frame clamp rows must be a contiguous prefix for a single TS op
    assert clamp_main == list(range(len(clamp_main))), \
        f"rev main frames not a prefix: {clamp_main}"
    n_clamp_main = len(clamp_main)

    # u/w op groups over x rows (device op planning):
    #  u: Sin for rev frames, Copy for prismatic ones, per contiguous run
    u_sin = [(u_row[n], xrow[n], int(types[n]) == 1) for n in var_frames]
    w_sin = [(w_row[n], xrow[n]) for n in rev_frames]

    host = dict(
        nf=nf, dof=dof, nrows=nrows, nx=nx,
        T16=np.ascontiguousarray(T.reshape(nrows, nf * 12).astype(np.float16)),
        multcol=multcol.astype(np.float32),
        extra_specs=extra_specs,
        n_clamp_main=n_clamp_main,
        u_sin=u_sin, w_sin=w_sin,
    )
    return host


# --------------------------------------------------------------------------
# Device program
# --------------------------------------------------------------------------

def _runs(items):
    """Group (dst_row, src_row, flag) triples into contiguous runs."""
    runs = []
    for d, s, f in items:
        if runs and runs[-1][0] + runs[-1][2] == d and \
                runs[-1][1] + runs[-1][2] == s and runs[-1][3] == f:
            runs[-1][2] += 1
        else:
            runs.append([d, s, 1, f])
    return runs


def _build_program(b_core, host):
    nf = host["nf"]
    dof = host["dof"]
    nrows = host["nrows"]
    nx = host["nx"]
    assert b_core % P == 0
    q = b_core // P
    chi = int(os.environ.get("FK_CHI", "53"))
    chi = max(0, min(q, chi))
    pi = float(np.pi)

    nc = bacc.Bacc("TRN2", target_bir_lowering=False, debug=False)

    theta_d = nc.dram_tensor("theta", [b_core, dof], F32,
                             kind="ExternalInput").ap()
    tabs_d = nc.dram_tensor("tabs", [nrows, nf * 12], F16,
                            kind="ExternalInput").ap()
    mult_d = nc.dram_tensor("multc", [P, dof], F32, kind="ExternalInput").ap()
    out_d = nc.dram_tensor("out", [nf, P, 12, q], F16,
                           kind="ExternalOutput").ap()

    theta_v = theta_d.rearrange("(p q) d -> p q d", p=P)

    from contextlib import ExitStack

    with tile.TileContext(nc) as tc, ExitStack() as ctx:
        cpool = ctx.enter_context(tc.tile_pool(name="const", bufs=1))
        ident = cpool.tile([P, P], F16)
        masks.make_identity(nc, ident[:])
        halfpi = cpool.tile([P, 1], F32)
        nc.vector.memset(halfpi[:], pi / 2.0)

        persist = ctx.enter_context(tc.tile_pool(name="persist", bufs=1))
        uw = persist.tile([P, nrows, q], F16)
        coefT = persist.tile([nrows, q, P], F16)
        tab_t = persist.tile([nrows, nf * 12], F16)
        # L chunks: frame halves; the chain starts once half 0 is evacuated.
        fhalf = nf // 2
        lchunks = [(0, fhalf), (fhalf, nf)]
        l16c = [persist.tile([P, f1 - f0, 12, q], F16, tag=f"l16{i}",
                             name=f"l16{i}") for i, (f0, f1) in
                enumerate(lchunks)]

        apool = ctx.enter_context(tc.tile_pool(name="coef", bufs=1))
        theta_t = apool.tile([P, q, dof], F32)
        mult_t = apool.tile([P, dof], F32)
        x32 = apool.tile([P, nx, q], F32)

        qh = q // 2
        nc.sync.dma_start(tab_t[:], tabs_d)
        nc.sync.dma_start(theta_t[:, :qh, :], theta_v[:, :qh, :])
        nc.sync.dma_start(mult_t[:], mult_d)
        nc.sync.dma_start(theta_t[:, qh:, :], theta_v[:, qh:, :])

        nhalf = nf * 12 // 2          # rhs columns per half (=384)
        tpp = ctx.enter_context(
            tc.tile_pool(name="tp_psum", bufs=2, space="PSUM"))
        lpp = ctx.enter_context(
            tc.tile_pool(name="l_psum", bufs=3, space="PSUM"))
        # W state in frame pairs: one DMA per 2 frames (fewer sync points)
        wpool = ctx.enter_context(tc.tile_pool(name="wpool", bufs=3))

        engs = []
        if chi > 0:
            engs.append((nc.vector, slice(0, chi)))
        if chi < q:
            engs.append((nc.gpsimd, slice(chi, q)))
        pools = [ctx.enter_context(tc.tile_pool(name=f"pp{ei}", bufs=2))
                 for ei in range(len(engs))]

        def levac(dst, src, e):
            # GPSIMD cannot access PSUM on HW: DVE/Act only.
            if e == 0:
                nc.vector.tensor_copy(dst, src)
            else:
                nc.scalar.copy(dst, src)

        # weighted engine pattern for half-0 L evacs (Act-heavy: Act has
        # the least other work at this point)
        pat = [1, 0, 1, 0, 1, 1, 0, 1, 0, 1, 1, 0, 1, 0, 1, 1]

        def bchunk(ci, qq):
            # matmul chunk ci for column pair (qq, qq+1) + one paired evac
            f0, f1 = lchunks[ci]
            ncols = (f1 - f0) * 12
            lp = lpp.tile([P, 2, 512], F32, tag="lp")
            for c in range(2):
                nc.tensor.matmul(lp[:, c, :ncols], coefT[:, qq + c, :],
                                 tab_t[:, f0 * 12:f1 * 12],
                                 start=True, stop=True)
            levac(l16c[ci][:, :, :, qq:qq + 2]
                  .rearrange("p f e c -> p c f e"),
                  lp[:, :, :ncols].rearrange("p c (f e) -> p c f e", e=12),
                  pat[(qq // 2) % len(pat)] if ci == 0 else 1)

        def b1(qq0, qq1):
            # per column pair: 2 transposes -> one coefT evac (DVE/Act) ->
            # chunk-0 matmuls -> one paired L evac. Chunk 1 is emitted
            # separately so every column's chunk 0 (the chain gate) is first.
            for qq in range(qq0, qq1, 2):
                tp = tpp.tile([nrows, 2, P], F16, tag="tp")
                nc.tensor.transpose(tp[:, 0, :], uw[:, :, qq], ident[:])
                nc.tensor.transpose(tp[:, 1, :], uw[:, :, qq + 1], ident[:])
                if (qq // 2) % 2 == 0:
                    nc.vector.tensor_copy(coefT[:, qq:qq + 2, :], tp[:])
                else:
                    nc.scalar.copy(coefT[:, qq:qq + 2, :], tp[:])
                bchunk(0, qq)

        ncl = host["n_clamp_main"]
        # w-row runs map directly x-row -> uw row (Abs is done in place)
        wdr = _runs([(d, s, 0) for d, s in host["w_sin"]])

        def emit_coef_half(h, hs):
            thT = theta_t[:, hs, :].rearrange("p q d -> p d q")
            mb = mult_t[:].unsqueeze(2).broadcast_to([P, dof, qh])
            eng = nc.vector if h == 0 else nc.gpsimd
            eng.tensor_mul(x32[:, :dof, hs], thT, mb)
            for xr, s, m, o in host["extra_specs"]:
                eng.tensor_scalar(x32[:, xr, hs], theta_t[:, hs, s],
                                  float(m), float(o),
                                  op0=OP.mult, op1=OP.add)
            # clamp rev rows (main prefix + extras, all rev here)
            eng.tensor_scalar(x32[:, :ncl, hs], x32[:, :ncl, hs],
                              pi, -pi, op0=OP.min, op1=OP.max)
            if nx > dof:
                eng.tensor_scalar(x32[:, dof:nx, hs], x32[:, dof:nx, hs],
                                  pi, -pi, op0=OP.min, op1=OP.max)

            # u rows: Sin for rev, Copy for prismatic (contiguous runs)
            for d0, s0, cnt, is_rev in _runs(host["u_sin"]):
                if is_rev:
                    nc.scalar.activation(uw[:, d0:d0 + cnt, hs],
                                         x32[:, s0:s0 + cnt, hs], AF.Sin)
                else:
                    nc.scalar.copy(uw[:, d0:d0 + cnt, hs],
                                   x32[:, s0:s0 + cnt, hs])
            # w rows: cos(x) = Sin(pi/2 - |x|); |x| in place via Act Abs
            for d0, s0, cnt, _f in wdr:
                nc.scalar.activation(x32[:, s0:s0 + cnt, hs],
                                     x32[:, s0:s0 + cnt, hs], AF.Abs)
            for d0, s0, cnt, _f in wdr:
                nc.scalar.activation(uw[:, d0:d0 + cnt, hs],
                                     x32[:, s0:s0 + cnt, hs], AF.Sin,
                                     bias=halfpi[:, 0:1], scale=-1.0)

        def emit_body():
            nc.vector.memset(uw[:, 0, :], 1.0)
            # ---------- Phase A + B1 (pipelined in q-halves) --------------
            for h, hs in enumerate((slice(0, qh), slice(qh, q))):
                emit_coef_half(h, hs)
                if h == 0:
                    b1(0, qh)
            b1(qh, q)

            # ---------- Phase B2: second frame-half chunks ----------------
            for qq in range(0, q, 2):
                bchunk(1, qq)

            emit_chain()

        def emit_chain():
            # ---------- Phase C: chain (q-sliced over DVE/Pool) -----------
            def lrow(n, k, s):
                ci = n // fhalf
                lh = l16c[ci]
                return lh[:, n - lchunks[ci][0], 4 * k:4 * k + 4, s] \
                    .unsqueeze(1).broadcast_to([P, 3, 4, s.stop - s.start])

            nc.sync.dma_start(out_d[0], l16c[0][:, 0, :, :])

            w_prev = [l16c[0][:, 0, :, s].rearrange("p (k j) q -> p k j q",
                                                    j=4)
                      for _e, s in engs]
            wpair = None
            for n in range(1, nf):
                if wpair is None:
                    wpair = wpool.tile([P, 2, 3, 4, q], F16, tag="W")
                    pn = 0
                w_new = wpair[:][:, pn]
                for ei, (eng, s) in enumerate(engs):
                    ql = s.stop - s.start
                    pp = pools[ei]
                    p0 = pp.tile([P, 3, 4, ql], F16, tag=f"p0{ei}")
                    p1 = pp.tile([P, 3, 4, ql], F16, tag=f"p1{ei}")
                    wpv = w_prev[ei]
                    wns = w_new[:, :, :, s]
                    wc = [wpv[:, :, k, :].unsqueeze(2)
                          .broadcast_to([P, 3, 4, ql]) for k in range(4)]
                    lr = [lrow(n, k, s) for k in range(3)]
                    eng.tensor_mul(p0[:], wc[0], lr[0])
                    eng.tensor_mul(p1[:], wc[1], lr[1])
                    eng.tensor_add(p0[:], p0[:], p1[:])
                    # t-col: += W_prev t-col folded in before the final add
                    eng.tensor_add(p0[:][:, :, 3, :],
                                   p0[:][:, :, 3, :], wpv[:, :, 3, :])
                    eng.tensor_mul(p1[:], wc[2], lr[2])
                    eng.tensor_add(wns, p0[:], p1[:])
                    w_prev[ei] = w_new[:, :, :, s]
                if pn == 1 or n == nf - 1:
                    nfr = pn + 1
                    nc.sync.dma_start(
                        out_d[n - pn:n + 1].rearrange("c p e q -> p c e q"),
                        wpair[:][:, :nfr]
                        .rearrange("p c k j q -> p c (k j) q"))
                    wpair = None
                else:
                    pn = 1

        reps = int(os.environ.get("FK_REPS", "1"))
        phase = os.environ.get("FK_PHASE", "all")
        if phase != "all":
            # phase-isolated builds for HW timing diagnosis
            def emit_coef_only():
                nc.vector.memset(uw[:, 0, :], 1.0)
                for h, hs in enumerate((slice(0, qh), slice(qh, q))):
                    emit_coef_half(h, hs)

            def emit_b_only():
                b1(0, q)
                for qq in range(0, q, 2):
                    bchunk(1, qq)

            def emit_chain_only():
                emit_chain()

            sel = {"coef": [emit_coef_only],
                   "lbuild": [emit_coef_only, emit_b_only],
                   "chain": [emit_chain_only]}[phase]
            if phase == "chain":
                for lc in l16c:
                    nc.vector.memset(lc[:], 0.25)
            for _rep in range(reps):
                for f in sel:
                    f()
        else:
            for _rep in range(reps):
                emit_body()

    nc.compile()
    return nc


def _get_program(b_core, host):
    key = (b_core, host["nrows"], os.environ.get("FK_CHI", "53"),
           os.environ.get("FK_REPS", "1"))
    prog = _program_cache.get(key)
    if prog is None:
        prog = _build_program(b_core, host)
        _program_cache[key] = prog
    return prog


# --------------------------------------------------------------------------
# Entry point
# --------------------------------------------------------------------------

def kernel(joint_angles, all_axes, all_origins, mimic_multipliers,
           mimic_offsets, ctrlable_indices, mimic_dst_indices,
           mimic_src_indices, joint_types):
    global last_results

    theta = np.ascontiguousarray(np.asarray(joint_angles, dtype=np.float32))
    batch, dof = theta.shape
    nf = np.asarray(all_axes).shape[0]

    host = _fold(all_axes, all_origins, mimic_multipliers, mimic_offsets,
                 ctrlable_indices, mimic_dst_indices, mimic_src_indices,
                 joint_types)

    assert batch % N_CORES == 0
    b_core = batch // N_CORES
    q = b_core // P

    nc = _get_program(b_core, host)

    multc = np.ascontiguousarray(
        np.broadcast_to(host["multcol"][None], (P, dof)).astype(np.float32))

    in_maps = []
    for i in range(N_CORES):
        in_maps.append({
            "theta": np.ascontiguousarray(theta[i * b_core:(i + 1) * b_core]),
            "tabs": host["T16"],
            "multc": multc,
        })

    global last_in_maps
    last_in_maps = in_maps
    res = run_bass_kernel_spmd(nc, in_maps, core_ids=list(range(N_CORES)))
    last_results = res

    full = np.empty((batch, nf, 4, 4), dtype=np.float32)
    for i in range(N_CORES):
        o = res.results[i]["out"].reshape(nf, P, 12, q)
        arr = np.ascontiguousarray(o.transpose(1, 3, 0, 2)).astype(np.float32)
        full[i * b_core:(i + 1) * b_core, :, :3, :] = \
            arr.reshape(b_core, nf, 3, 4)
    full[:, :, 3, :3] = 0.0
    full[:, :, 3, 3] = 1.0
    return full
